# revision 35
# baseline (speedup 1.0000x reference)
"""MoE routing kernel for TRN2, 8 NeuronCores (expert-parallel).

Math: out[t] = sum_{e in top2(logits[t])} x[t] @ w_up[e] @ w_down[e]
(reference applies no activation between projections and no prob weighting,
so each expert collapses to one fused matrix W_e = w_up[e] @ w_down[e]).

Schedule (v2.2) — built around two measured facts: the CC stream is
unavailable until the runtime init barrier completes (~70us), and each
AllGather costs ~(20us floor + bytes/230GBps) serialized on that stream.

  Host prep: weights cast to bf16, x/gate transposed on host.
  Phase 1: core c computes W_c = up_c @ down_c in three row-stages
    (384/384/256 rows). Stage 0's matmuls are emitted BEFORE the router so
    the PE starts as soon as the first weight quarter lands (~15us). After
    each stage: DVE evac -> bf16 -> AllGather of that row slice. Gathered
    slices stream back on the scalar+vector DMA queues (the sync queue
    carries x/upT, gpsimd only the tiny maskT writes + AG triggers).
  Phase 2: router (fp32, exact top-2) + masked x^T copies per expert; apply
    accumulates all 8 experts x 8 d1-tiles into 4x[128,512] PSUM regions.
"""

import numpy as np

E = 8
TOPK = 2
D = 1024
I = 4096
T = 2048  # 4*512 tokens
N_CORES = 8
TL = T // N_CORES  # tokens per core (256)
P = 128
BIG = 1.0e30

MQS = [3, 2, 3]   # m-tiles (128 W rows each) per stage
NQ = len(MQS)
MOFF = [0, 3, 5]  # first m-tile of each stage
KB = D // P       # 8  d1 tiles
KI = I // P       # 32 contraction tiles
TB = TL // P      # 2  token tiles

_cached = {}


def _build():
    import concourse.bass as bass  # noqa: F401
    import concourse.tile as tile
    from concourse import bacc, mybir
    from concourse.masks import make_identity

    f32 = mybir.dt.float32
    bf16 = mybir.dt.bfloat16

    nc = bacc.Bacc("TRN2", target_bir_lowering=False, debug=False, num_devices=N_CORES)
    xT_ext = nc.declare_dram_parameter("xT", [D, TL], f32, isOutput=False)
    gateT_ext = nc.declare_dram_parameter("gateT", [D, E], f32, isOutput=False)
    upT_ext = nc.declare_dram_parameter("w_upT", [I, D], bf16, isOutput=False)
    down_ext = nc.declare_dram_parameter("w_down", [I, D], bf16, isOutput=False)
    out_ext = nc.declare_dram_parameter("out", [TL, D], f32, isOutput=True)

    with tile.TileContext(nc) as tc:
        import contextlib

        with contextlib.ExitStack() as ctx:
            const = ctx.enter_context(tc.tile_pool(name="const", bufs=1))
            dram = ctx.enter_context(tc.tile_pool(name="dram", bufs=1, space="DRAM"))
            xmp = ctx.enter_context(tc.tile_pool(name="xm", bufs=1))

            ident = const.tile([P, P], f32)
            make_identity(nc, ident[:])
            ones1 = const.tile([1, P], bf16)
            nc.vector.memset(ones1[:], 1.0)

            ag_in = []
            ag_out = []
            for q in range(NQ):
                gi = dram.tile([P, MQS[q] * D], bf16, name=f"ag_in_{q}")
                go = dram.tile(
                    [E, P, MQS[q] * D], bf16, addr_space="Shared", name=f"ag_out_{q}"
                )
                ag_in.append(gi)
                ag_out.append(go)

            xms = []
            for e in range(E):
                xm = xmp.tile([P, KB, TL], bf16, tag=f"xm{e}", name=f"xm_{e}")
                xms.append(xm)

            wevp = ctx.enter_context(tc.tile_pool(name="wev", bufs=1))
            wep = ctx.enter_context(tc.tile_pool(name="wep", bufs=6))
            wes = [[None] * E for _ in range(NQ)]

            # ---- weight DMA first: upq stage-0 quarter-0, then x/gate, rest ----
            up_cm = tc.tile_pool(name="up", bufs=4)
            up = up_cm.__enter__()
            upq = [[None] * 4 for _ in range(NQ)]  # [q][quarter] -> [P, 8, 128*MQS[q]]

            def load_upq(q, h):
                t = up.tile(
                    [P, KI // 4, P * MQS[q]], bf16, tag="upq", name=f"upq_{q}_{h}"
                )
                nc.sync.dma_start(
                    t[:],
                    upT_ext[
                        1024 * h : 1024 * (h + 1),
                        P * MOFF[q] : P * (MOFF[q] + MQS[q]),
                    ].rearrange("(ko p) n -> p ko n", p=P),
                )
                upq[q][h] = t

            for h in range(4):
                load_upq(0, h)

            rt_cm = tc.tile_pool(name="router", bufs=1)
            rt = rt_cm.__enter__()
            xT = rt.tile([P, KB, TL], f32)
            nc.sync.dma_start(xT[:], xT_ext.rearrange("(kb p) t -> p kb t", p=P))
            gateT = rt.tile([P, KB, E], f32)
            nc.sync.dma_start(gateT[:], gateT_ext.rearrange("(kb p) e -> p kb e", p=P))

            for q in range(1, NQ):
                for h in range(4):
                    load_upq(q, h)

            dn_cm = tc.tile_pool(name="dn", bufs=1)
            dn = dn_cm.__enter__()
            down = dn.tile([P, KI, D], bf16)
            for g in range(8):
                nc.scalar.dma_start(
                    down[:, 4 * g : 4 * (g + 1), :],
                    down_ext[512 * g : 512 * (g + 1), :].rearrange(
                        "(ko p) n -> p ko n", p=P
                    ),
                )

            psA_cm = tc.tile_pool(name="psA", bufs=2, space="PSUM")
            psA = psA_cm.__enter__()

            logits = rt.tile([P, TB, E], f32)
            m1 = rt.tile([P, TB], f32)
            eqbig = rt.tile([P, TB, E], f32)
            l2 = rt.tile([P, TB, E], f32)
            m2 = rt.tile([P, TB], f32)
            mask = rt.tile([P, TB, E], f32)
            mtmp = rt.tile([8, TB, P], bf16)
            maskT = rt.tile([1, E, TL], bf16)
            mbc = rt.tile([P, E, TL], f32)

            def w_stage(q, psW):
                mq = MQS[q]
                pw = [
                    [
                        psW.tile([P, 512], f32, tag=f"pw{m2}{ch}", name=f"pw_{q}_{m2}_{ch}")
                        for ch in range(2)
                    ]
                    for m2 in range(mq)
                ]
                for k in range(KI):
                    lhs = upq[q][k // 8]
                    for m2 in range(mq):
                        for ch in range(2):
                            nc.tensor.matmul(
                                pw[m2][ch][:],
                                lhs[:, k % 8, P * m2 : P * (m2 + 1)],
                                down[:, k, 512 * ch : 512 * (ch + 1)],
                                start=(k == 0),
                                stop=(k == KI - 1),
                            )
                wev = wevp.tile([P, mq, D], bf16, tag="wev", name=f"wev_{q}")
                for m2 in range(mq):
                    for ch in range(2):
                        nc.vector.tensor_copy(
                            out=wev[:, m2, 512 * ch : 512 * (ch + 1)],
                            in_=pw[m2][ch][:],
                        )
                return wev

            def ag_stage(q, wev, eng):
                mq = MQS[q]
                eng.dma_start(ag_in[q][:].rearrange("p (m n) -> p m n", m=mq), wev[:])
                nc.gpsimd.collective_compute(
                    "AllGather",
                    mybir.AluOpType.bypass,
                    replica_groups=[list(range(N_CORES))],
                    ins=[ag_in[q].opt()],
                    outs=[ag_out[q].opt()],
                )

            def we_loads(q):
                mq = MQS[q]
                for e in range(E):
                    we = wep.tile([P, mq, D], bf16, tag="we", name=f"we_{q}_{e}")
                    eng = nc.scalar if e % 2 == 0 else nc.sync
                    eng.dma_start(
                        we[:], ag_out[q][e].rearrange("p (m n) -> p m n", m=mq)
                    )
                    wes[q][e] = we

            # ---- stage 0 matmuls first: PE starts ~15us in ----
            psW_cm = tc.tile_pool(name="psW0", bufs=1, space="PSUM")
            wev0 = w_stage(0, psW_cm.__enter__())

            # router logits (fp32 exact): logits[t, e] = x @ gate_w.T
            for tb in range(TB):
                pl = psA.tile([P, E], f32, tag="tp")
                for kb in range(KB):
                    nc.tensor.matmul(
                        pl[:],
                        xT[:, kb, P * tb : P * (tb + 1)],
                        gateT[:, kb, :],
                        start=(kb == 0),
                        stop=(kb == KB - 1),
                    )
                nc.vector.tensor_copy(out=logits[:, tb, :], in_=pl[:])

            nc.vector.tensor_reduce(
                m1[:], logits[:], axis=mybir.AxisListType.X, op=mybir.AluOpType.max
            )
            nc.vector.tensor_tensor(
                eqbig[:],
                logits[:],
                m1[:, :, None].to_broadcast([P, TB, E]),
                mybir.AluOpType.is_equal,
            )
            nc.vector.tensor_scalar_mul(eqbig[:], eqbig[:], BIG)
            nc.vector.tensor_tensor(l2[:], logits[:], eqbig[:], mybir.AluOpType.subtract)
            nc.vector.tensor_reduce(
                m2[:], l2[:], axis=mybir.AxisListType.X, op=mybir.AluOpType.max
            )
            nc.vector.tensor_tensor(
                mask[:],
                logits[:],
                m2[:, :, None].to_broadcast([P, TB, E]),
                mybir.AluOpType.is_ge,
            )
            for tb in range(TB):
                pt = psA.tile([P, P], f32, tag="tp")
                nc.tensor.transpose(pt[:8, :], mask[:, tb, :], ident[:])
                nc.vector.tensor_copy(out=mtmp[:, tb, :], in_=pt[:8, :])
            for tb in range(TB):
                # gpsimd: tiny, and must precede the AG triggers on that engine
                nc.gpsimd.dma_start(maskT[0:1, :, P * tb : P * (tb + 1)], mtmp[:, tb, :])

            # AG of stage 0 (trigger sits after maskT on the gpsimd stream).
            # ag_in queues zigzag (scalar/sync/scalar) and each stage's we
            # loads are emitted a stage late, so a we load waiting on AG_q
            # never sits ahead of ag_in_{q+1} on the same queue.
            ag_stage(0, wev0, nc.scalar)
            psW_cm.__exit__(None, None, None)
            psA_cm.__exit__(None, None, None)

            psW1_cm = tc.tile_pool(name="psW1", bufs=1, space="PSUM")
            wev1 = w_stage(1, psW1_cm.__enter__())
            ag_stage(1, wev1, nc.sync)
            we_loads(0)
            psW1_cm.__exit__(None, None, None)

            # mbc/xm after stage 1: maskT (slow SWDGE write) has long landed,
            # so the outer-product matmuls cost ~1us of PE instead of a 20us
            # in-order stall; xm only needs to exist by the apply (~20us later)
            psA2_cm = tc.tile_pool(name="psA2", bufs=2, space="PSUM")
            psA2 = psA2_cm.__enter__()
            for e in range(E):
                pb = psA2.tile([P, TL], f32, tag="tp")
                nc.tensor.matmul(pb[:], ones1[:], maskT[0:1, e, :], start=True, stop=True)
                nc.vector.tensor_copy(out=mbc[:, e, :], in_=pb[:])
            for e in range(E):
                for kb in range(KB):
                    nc.vector.tensor_tensor(
                        xms[e][:, kb, :],
                        xT[:, kb, :],
                        mbc[:, e, :],
                        mybir.AluOpType.mult,
                    )
            psA2_cm.__exit__(None, None, None)

            psW2_cm = tc.tile_pool(name="psW2", bufs=1, space="PSUM")
            wev2 = w_stage(2, psW2_cm.__enter__())
            ag_stage(2, wev2, nc.scalar)
            we_loads(1)
            we_loads(2)
            psW2_cm.__exit__(None, None, None)

            dn_cm.__exit__(None, None, None)
            rt_cm.__exit__(None, None, None)
            up_cm.__exit__(None, None, None)

            # ---- phase 2: apply ----
            ap_cm = tc.tile_pool(name="apply", bufs=1)
            ap = ap_cm.__enter__()
            psO_cm = tc.tile_pool(name="psO", bufs=1, space="PSUM")
            psO = psO_cm.__enter__()

            pout = [
                [
                    psO.tile([P, 512], f32, tag=f"o{tt}{ch}", name=f"pout_{tt}_{ch}")
                    for ch in range(2)
                ]
                for tt in range(TB)
            ]
            for q in range(NQ):
                for e in range(E):
                    we = wes[q][e]
                    for kbq in range(MQS[q]):
                        kb = MOFF[q] + kbq
                        for tt in range(TB):
                            for ch in range(2):
                                nc.tensor.matmul(
                                    pout[tt][ch][:],
                                    xms[e][:, kb, P * tt : P * (tt + 1)],
                                    we[:, kbq, 512 * ch : 512 * (ch + 1)],
                                    start=(q == 0 and e == 0 and kbq == 0),
                                    stop=(
                                        q == NQ - 1
                                        and e == E - 1
                                        and kbq == MQS[q] - 1
                                    ),
                                )

            outsb = ap.tile([P, TB, D], f32, tag="outsb")
            for tt in range(TB):
                for ch in range(2):
                    nc.vector.tensor_copy(
                        out=outsb[:, tt, 512 * ch : 512 * (ch + 1)],
                        in_=pout[tt][ch][:],
                    )
            nc.sync.dma_start(out_ext.rearrange("(b p) d -> p b d", p=P), outsb[:])

            psO_cm.__exit__(None, None, None)
            ap_cm.__exit__(None, None, None)

    nc.finalize()
    return nc


def _get_nc():
    if "nc" not in _cached:
        _cached["nc"] = _build()
    return _cached["nc"]


def _make_in_maps(inputs):
    import ml_dtypes

    bf16 = ml_dtypes.bfloat16
    hs = np.asarray(inputs["hidden_states"], dtype=np.float32)
    gate_w = np.asarray(inputs["gate_w"], dtype=np.float32)
    w_up = np.asarray(inputs["w_up"], dtype=np.float32)
    w_down = np.asarray(inputs["w_down"], dtype=np.float32)
    x = hs.reshape(-1, D)
    gateT = np.ascontiguousarray(gate_w.T)
    in_maps = []
    for c in range(N_CORES):
        in_maps.append(
            {
                "xT": np.ascontiguousarray(x[TL * c : TL * (c + 1)].T),
                "gateT": gateT,
                "w_upT": np.ascontiguousarray(w_up[c].T).astype(bf16),
                "w_down": np.ascontiguousarray(w_down[c]).astype(bf16),
            }
        )
    return in_maps, hs.shape


def kernel(**inputs) -> np.ndarray:
    from concourse.bass_utils import run_bass_kernel_spmd

    in_maps, orig_shape = _make_in_maps(inputs)
    nc = _get_nc()
    last_err = None
    for _attempt in range(3):
        try:
            res = run_bass_kernel_spmd(nc, in_maps, core_ids=list(range(N_CORES)))
            break
        except Exception as err:  # transient NRT/device hiccup: retry
            last_err = err
            import time as _time

            _time.sleep(2.0)
    else:
        raise last_err
    out = np.concatenate([res.results[c]["out"] for c in range(N_CORES)], axis=0)
    return out.reshape(orig_shape).astype(np.float32)


def run_traced(**inputs):
    """Like kernel() but returns (out, BassKernelResults with trace)."""
    from concourse.bass_utils import run_bass_kernel_spmd

    in_maps, orig_shape = _make_in_maps(inputs)
    nc = _get_nc()
    res = run_bass_kernel_spmd(nc, in_maps, core_ids=list(range(N_CORES)), trace=True)
    out = np.concatenate([res.results[c]["out"] for c in range(N_CORES)], axis=0)
    return out.reshape(orig_shape).astype(np.float32), res
